# revision 12
# baseline (speedup 1.0000x reference)
"""MoE routing kernel for Trainium2 (8 NeuronCores, expert-parallel dispatch).

Reference computes, per token t (T = 4096 tokens, D = 1024, E = 8, H = 4096):
  logits = x @ router_w + router_b
  e*(t)  = argmax_e softmax(logits + gumbel)      (hard gumbel-softmax)
  out[t] = c_t * (gelu(x[t] @ w1[e*] + b1[e*]) @ w2[e*] + b2[e*])
where the straight-through weights are exactly one-hot in the forward pass
(c_t = fl(fl(1 - y_sel) + y_sel) ~ 1), plus a scalar load-balancing aux loss.

Strategy: the router (67 MFLOP) runs on host as part of sharding; tokens are
dispatched to their expert's core (expert-parallel, one expert per core). Each
core runs a dense bf16 MLP over its gathered tokens, padded to a common
capacity C. Everything stays transposed ([feature, token] layout) so no
on-device transposes are needed:
  hiddenT[h, j] = gelu(sum_d W1[d, h] * XT[d, j] + b1[h])   (lhsT = W1 tiles)
  outT[d, j]   = sum_h W2[h, d] * hiddenT[h, j] + b2[d]     (lhsT = W2 tiles)
Host scatters per-core outputs back to token order and applies c_t.
"""

import math

import numpy as np

import concourse.bacc as bacc
import concourse.mybir as mybir
import concourse.tile as tile
from concourse import bass_utils

B, N, D, E, H = 4, 1024, 1024, 8, 4096
T = B * N
P = 128
ND = D // P   # 8 d-tiles
NH = H // P   # 32 h-tiles
CSEG_MAX = 1024  # max padded tokens per core per launch (SBUF-bounded)

_FP = mybir.dt.float32
_BF = mybir.dt.bfloat16
_BF_NP = mybir.dt.np(_BF)

_nc_cache = {}


def _build(C):
    """Dense transposed MLP over C padded tokens: yT = W2^T gelu(W1^T xT + b1) + b2."""
    nc = bacc.Bacc("TRN2", target_bir_lowering=False, debug=False, num_devices=E)
    NB = NH // 8  # w2 DMA blocks of 8 h-tiles
    xt_d = nc.dram_tensor("xt", (ND, P, C), _BF, kind="ExternalInput")
    w1t_d = nc.dram_tensor("w1t", (NH, P, ND, P), _BF, kind="ExternalInput")
    b1r_d = nc.dram_tensor("b1r", (P, NH), _FP, kind="ExternalInput")
    w2t_d = nc.dram_tensor("w2t", (NB, P, 8, D), _BF, kind="ExternalInput")
    b2r_d = nc.dram_tensor("b2r", (P, ND), _FP, kind="ExternalInput")
    yt_d = nc.dram_tensor("yt", (ND, P, C), _FP, kind="ExternalOutput")

    # token-dim chunks, each <= 512 so a psum accumulator fits one bank;
    # equal-width chunks keep every matmul above the ~56ns issue floor
    nchunk = math.ceil(C / 512)
    csz = 16 * math.ceil(C / nchunk / 16)
    chunks = [(c0, min(csz, C - c0)) for c0 in range(0, C, csz)]

    with tile.TileContext(nc) as tc:
        with (
            tc.tile_pool(name="xpool", bufs=1) as xpool,
            tc.tile_pool(name="w1pool", bufs=6) as w1pool,
            tc.tile_pool(name="w2pool", bufs=1) as w2pool,
            tc.tile_pool(name="hpool", bufs=1) as hpool,
            tc.tile_pool(name="bpool", bufs=1) as bpool,
            tc.tile_pool(name="opool", bufs=4) as opool,
            tc.tile_pool(name="pspool", bufs=3, space="PSUM") as pspool,
        ):
            xt_sb = xpool.tile([P, ND, C], _BF)    # resident: xT
            h_sb = hpool.tile([P, NH, C], _BF)     # resident: hiddenT
            w2_sb = w2pool.tile([P, NH, D], _BF)   # resident: all of W2
            b1_sb = bpool.tile([P, NH], _FP)
            b2_sb = bpool.tile([P, ND], _FP)

            nc.sync.dma_start(b1_sb[:], b1r_d[:])
            nc.sync.dma_start(b2_sb[:], b2r_d[:])
            # stream x and the first w1 tile per-dt so the first matmul chain
            # can start as soon as its own dt pieces have landed
            w1_first = w1pool.tile([P, ND, P], _BF, tag="w1_sb")
            for dt in range(ND):
                nc.sync.dma_start(w1_first[:, dt, :], w1t_d[0][:, dt, :])
                nc.sync.dma_start(xt_sb[:, dt, :], xt_d[dt])

            # phase 1: hiddenT[ht] = gelu(sum_dt W1[dt,ht]^T @ xT[dt] + b1[ht])
            # w2 streams in alongside so it's resident by the time phase 2 runs
            for ht in range(NH):
                if ht == 0:
                    w1_sb = w1_first
                else:
                    w1_sb = w1pool.tile([P, ND, P], _BF, tag="w1_sb")
                    nc.sync.dma_start(w1_sb[:], w1t_d[ht])
                if ht % 8 == 4:
                    b = ht // 8
                    nc.sync.dma_start(w2_sb[:, 8 * b : 8 * b + 8, :], w2t_d[b])
                for c0, cw in chunks:
                    ps = pspool.tile([P, cw], _FP, tag=f"ps{cw}")
                    for dt in range(ND):
                        nc.tensor.matmul(
                            ps[:],
                            w1_sb[:, dt, :],
                            xt_sb[:, dt, c0 : c0 + cw],
                            start=(dt == 0),
                            stop=(dt == ND - 1),
                        )
                    nc.scalar.activation(
                        h_sb[:, ht, c0 : c0 + cw],
                        ps[:],
                        mybir.ActivationFunctionType.Gelu,
                        bias=b1_sb[:, ht : ht + 1],
                    )

            # phase 2: outT[dt] = sum_ht W2[ht,dt]^T @ hiddenT[ht] + b2[dt]
            for dt in range(ND):
                for c0, cw in chunks:
                    ps = pspool.tile([P, cw], _FP, tag=f"ps{cw}")
                    for ht in range(NH):
                        nc.tensor.matmul(
                            ps[:],
                            w2_sb[:, ht, dt * P : (dt + 1) * P],
                            h_sb[:, ht, c0 : c0 + cw],
                            start=(ht == 0),
                            stop=(ht == NH - 1),
                        )
                    ot = opool.tile([P, cw], _FP, tag=f"o{cw}")
                    nc.vector.tensor_scalar_add(ot[:], ps[:], b2_sb[:, dt : dt + 1])
                    nc.sync.dma_start(yt_d[dt][:, c0 : c0 + cw], ot[:])

    nc.compile()
    return nc


def _get_nc(C):
    if C not in _nc_cache:
        _nc_cache[C] = _build(C)
    return _nc_cache[C]


def _route(xf, gumbel, router_w, router_b):
    """Host-side router, mirroring the reference's f32 op order."""
    logits = xf @ router_w + router_b
    z = logits + gumbel
    ez = np.exp(z - z.max(axis=1, keepdims=True), dtype=np.float32)
    am = ez.argmax(axis=1)  # == argmax of softmax (incl. tie-break on first)
    ysel = ez[np.arange(T), am] / ez.sum(axis=1)
    coef = (np.float32(1.0) - ysel) + ysel  # straight-through weight at e*

    el = np.exp(logits - logits.max(axis=1, keepdims=True), dtype=np.float32)
    probs = el / el.sum(axis=1, keepdims=True)
    importance = probs.sum(axis=0, dtype=np.float32) / np.float32(T)
    diff = importance - np.float32(1.0 / E)
    aux = np.float32(np.mean(diff * diff, dtype=np.float32))
    return am, coef, aux


def _run(x, gumbel, router_w, router_b, w1, b1, w2, b2, trace=False, trace_cores=None):
    x = np.ascontiguousarray(np.asarray(x, dtype=np.float32))
    gumbel = np.asarray(gumbel, dtype=np.float32)
    router_w = np.asarray(router_w, dtype=np.float32)
    router_b = np.asarray(router_b, dtype=np.float32)
    w1 = np.asarray(w1, dtype=np.float32)
    b1 = np.asarray(b1, dtype=np.float32)
    w2 = np.asarray(w2, dtype=np.float32)
    b2 = np.asarray(b2, dtype=np.float32)

    xf = x.reshape(T, D)
    am, coef, aux = _route(xf, gumbel, router_w, router_b)
    idx = [np.nonzero(am == e)[0] for e in range(E)]
    nmax = max(len(i) for i in idx)
    cap = max(64, 16 * math.ceil(nmax / 16))
    cseg = min(cap, CSEG_MAX)
    nseg = max(1, math.ceil(nmax / cseg))
    nc = _get_nc(cseg)

    # per-expert static operands (same across segments)
    w1t = [
        np.ascontiguousarray(
            w1[e].reshape(ND, P, NH, P).transpose(2, 1, 0, 3)
        ).astype(_BF_NP)
        for e in range(E)
    ]
    w2t = [
        np.ascontiguousarray(
            w2[e].reshape(NH // 8, 8, P, D).transpose(0, 2, 1, 3)
        ).astype(_BF_NP)
        for e in range(E)
    ]
    b1r = [np.ascontiguousarray(b1[e].reshape(NH, P).T) for e in range(E)]
    b2r = [np.ascontiguousarray(b2[e].reshape(ND, P).T) for e in range(E)]

    out = np.zeros((T, D), dtype=np.float32)
    last_results = None
    for s in range(nseg):
        in_maps = []
        for e in range(E):
            sel = idx[e][s * cseg : (s + 1) * cseg]
            xg = np.zeros((cseg, D), dtype=np.float32)
            xg[: len(sel)] = xf[sel]
            # xt[dt, p, j] = xg[j, dt*P + p] — contiguous per dt-tile
            xt = np.ascontiguousarray(xg.T.reshape(ND, P, cseg)).astype(_BF_NP)
            in_maps.append(
                {"xt": xt, "w1t": w1t[e], "b1r": b1r[e], "w2t": w2t[e], "b2r": b2r[e]}
            )
        res = bass_utils.run_bass_kernel_spmd(
            nc,
            in_maps,
            core_ids=list(range(E)),
            trace=trace,
            trace_cores=trace_cores,
        )
        last_results = res
        for e in range(E):
            sel = idx[e][s * cseg : (s + 1) * cseg]
            if len(sel) == 0:
                continue
            yg = res.results[e]["yt"].reshape(D, cseg).T
            out[sel] = yg[: len(sel)] * coef[sel, None]

    return out.reshape(B, N, D), aux, last_results


def kernel(x, gumbel, router_w, router_b, w1, b1, w2, b2):
    out, aux, _ = _run(x, gumbel, router_w, router_b, w1, b1, w2, b2)
    return out, aux


# revision 18
# speedup vs baseline: 1.0907x; 1.0907x over previous
"""MoE routing kernel for Trainium2 (8 NeuronCores, expert-parallel dispatch).

Reference computes, per token t (T = 4096 tokens, D = 1024, E = 8, H = 4096):
  logits = x @ router_w + router_b
  e*(t)  = argmax_e softmax(logits + gumbel)      (hard gumbel-softmax)
  out[t] = c_t * (gelu(x[t] @ w1[e*] + b1[e*]) @ w2[e*] + b2[e*])
where the straight-through weights are exactly one-hot in the forward pass
(c_t = fl(fl(1 - y_sel) + y_sel) ~ 1), plus a scalar load-balancing aux loss.

Strategy: the router (67 MFLOP) runs on host as part of sharding; tokens are
dispatched to their expert's core (expert-parallel, one expert per core). Each
core runs a dense bf16 MLP over its gathered tokens, padded to a common
capacity C. Everything stays transposed ([feature, token] layout) so no
on-device transposes are needed:
  hiddenT[h, j] = gelu(sum_d W1[d, h] * XT[d, j] + b1[h])   (lhsT = W1 tiles)
  outT[d, j]   = sum_h W2[h, d] * hiddenT[h, j] + b2[d]     (lhsT = W2 tiles)
Host scatters per-core outputs back to token order and applies c_t.
"""

import math

import numpy as np

import concourse.bacc as bacc
import concourse.mybir as mybir
import concourse.tile as tile
from concourse import bass_utils

B, N, D, E, H = 4, 1024, 1024, 8, 4096
T = B * N
P = 128
ND = D // P   # 8 d-tiles
NH = H // P   # 32 h-tiles
CSEG_MAX = 1024  # max padded tokens per core per launch (SBUF-bounded)

_FP = mybir.dt.float32
_BF = mybir.dt.bfloat16
_BF_NP = mybir.dt.np(_BF)

_nc_cache = {}


def _build(C):
    """Dense transposed MLP over C padded tokens: yT = W2^T gelu(W1^T xT + b1) + b2."""
    nc = bacc.Bacc("TRN2", target_bir_lowering=False, debug=False, num_devices=E)
    NB = NH // 8  # w2 DMA blocks of 8 h-tiles
    xt_d = nc.dram_tensor("xt", (P, ND, C), _BF, kind="ExternalInput")
    w1t_d = nc.dram_tensor("w1t", (NH, P, ND, P), _BF, kind="ExternalInput")
    b1r_d = nc.dram_tensor("b1r", (P, NH), _FP, kind="ExternalInput")
    w2t_d = nc.dram_tensor("w2t", (NB, P, 8, D), _BF, kind="ExternalInput")
    b2r_d = nc.dram_tensor("b2r", (P, ND), _FP, kind="ExternalInput")
    yt_d = nc.dram_tensor("yt", (ND, P, C), _FP, kind="ExternalOutput")

    # token-dim chunks, each <= 512 so a psum accumulator fits one bank
    chunks = [(c0, min(512, C - c0)) for c0 in range(0, C, 512)]

    with tile.TileContext(nc) as tc:
        with (
            tc.tile_pool(name="xpool", bufs=1) as xpool,
            tc.tile_pool(name="w1pool", bufs=6) as w1pool,
            tc.tile_pool(name="w2pool", bufs=1) as w2pool,
            tc.tile_pool(name="hpool", bufs=1) as hpool,
            tc.tile_pool(name="bpool", bufs=1) as bpool,
            tc.tile_pool(name="opool", bufs=4) as opool,
            tc.tile_pool(name="pspool", bufs=3, space="PSUM") as pspool,
        ):
            xt_sb = xpool.tile([P, ND, C], _BF)    # resident: xT
            h_sb = hpool.tile([P, NH, C], _BF)     # resident: hiddenT
            w2_sb = w2pool.tile([P, NH, D], _BF)   # resident: all of W2
            b1_sb = bpool.tile([P, NH], _FP)
            b2_sb = bpool.tile([P, ND], _FP)

            # PE pre-warm: junk matmuls on a zeroed tile while startup DMAs are
            # in flight, so HAM is at 2.4 GHz when the real chains begin
            warm = bpool.tile([P, P], _BF)
            nc.vector.memset(warm[:], 0.0)
            wps = pspool.tile([P, 16], _FP, tag="warm", bufs=1)
            for _ in range(48):
                nc.tensor.matmul(
                    wps[:], warm[:], warm[:, :16], start=True, stop=True
                )

            # first w1 tile first — it's the first thing PE needs
            w1_first = w1pool.tile([P, ND, P], _BF, tag="w1_sb")
            nc.sync.dma_start(w1_first[:], w1t_d[0])
            nc.sync.dma_start(xt_sb[:], xt_d[:])
            nc.sync.dma_start(b1_sb[:], b1r_d[:])
            nc.sync.dma_start(b2_sb[:], b2r_d[:])

            # phase 1: hiddenT[ht] = gelu(sum_dt W1[dt,ht]^T @ xT[dt] + b1[ht])
            # w2 streams in alongside so it's resident by the time phase 2 runs
            for ht in range(NH):
                if ht == 0:
                    w1_sb = w1_first
                else:
                    w1_sb = w1pool.tile([P, ND, P], _BF, tag="w1_sb")
                    nc.sync.dma_start(w1_sb[:], w1t_d[ht])
                if ht % 8 == 4:
                    b = ht // 8
                    nc.sync.dma_start(w2_sb[:, 8 * b : 8 * b + 8, :], w2t_d[b])
                for c0, cw in chunks:
                    ps = pspool.tile([P, cw], _FP, tag=f"ps{cw}")
                    for dt in range(ND):
                        nc.tensor.matmul(
                            ps[:],
                            w1_sb[:, dt, :],
                            xt_sb[:, dt, c0 : c0 + cw],
                            start=(dt == 0),
                            stop=(dt == ND - 1),
                        )
                    nc.scalar.activation(
                        h_sb[:, ht, c0 : c0 + cw],
                        ps[:],
                        mybir.ActivationFunctionType.Gelu,
                        bias=b1_sb[:, ht : ht + 1],
                    )

            # phase 2: outT[dt] = sum_ht W2[ht,dt]^T @ hiddenT[ht] + b2[dt]
            for dt in range(ND):
                for c0, cw in chunks:
                    ps = pspool.tile([P, cw], _FP, tag=f"ps{cw}")
                    for ht in range(NH):
                        nc.tensor.matmul(
                            ps[:],
                            w2_sb[:, ht, dt * P : (dt + 1) * P],
                            h_sb[:, ht, c0 : c0 + cw],
                            start=(ht == 0),
                            stop=(ht == NH - 1),
                        )
                    ot = opool.tile([P, cw], _FP, tag=f"o{cw}")
                    nc.vector.tensor_scalar_add(ot[:], ps[:], b2_sb[:, dt : dt + 1])
                    nc.sync.dma_start(yt_d[dt][:, c0 : c0 + cw], ot[:])

    nc.compile()
    return nc


def _get_nc(C):
    if C not in _nc_cache:
        _nc_cache[C] = _build(C)
    return _nc_cache[C]


def _route(xf, gumbel, router_w, router_b):
    """Host-side router, mirroring the reference's f32 op order."""
    logits = xf @ router_w + router_b
    z = logits + gumbel
    ez = np.exp(z - z.max(axis=1, keepdims=True), dtype=np.float32)
    am = ez.argmax(axis=1)  # == argmax of softmax (incl. tie-break on first)
    ysel = ez[np.arange(T), am] / ez.sum(axis=1)
    coef = (np.float32(1.0) - ysel) + ysel  # straight-through weight at e*

    el = np.exp(logits - logits.max(axis=1, keepdims=True), dtype=np.float32)
    probs = el / el.sum(axis=1, keepdims=True)
    importance = probs.sum(axis=0, dtype=np.float32) / np.float32(T)
    diff = importance - np.float32(1.0 / E)
    aux = np.float32(np.mean(diff * diff, dtype=np.float32))
    return am, coef, aux


def _run(x, gumbel, router_w, router_b, w1, b1, w2, b2, trace=False, trace_cores=None):
    x = np.ascontiguousarray(np.asarray(x, dtype=np.float32))
    gumbel = np.asarray(gumbel, dtype=np.float32)
    router_w = np.asarray(router_w, dtype=np.float32)
    router_b = np.asarray(router_b, dtype=np.float32)
    w1 = np.asarray(w1, dtype=np.float32)
    b1 = np.asarray(b1, dtype=np.float32)
    w2 = np.asarray(w2, dtype=np.float32)
    b2 = np.asarray(b2, dtype=np.float32)

    xf = x.reshape(T, D)
    am, coef, aux = _route(xf, gumbel, router_w, router_b)
    idx = [np.nonzero(am == e)[0] for e in range(E)]
    nmax = max(len(i) for i in idx)
    cap = max(64, 16 * math.ceil(nmax / 16))
    cseg = min(cap, CSEG_MAX)
    nseg = max(1, math.ceil(nmax / cseg))
    nc = _get_nc(cseg)

    # per-expert static operands (same across segments)
    w1t = [
        np.ascontiguousarray(
            w1[e].reshape(ND, P, NH, P).transpose(2, 1, 0, 3)
        ).astype(_BF_NP)
        for e in range(E)
    ]
    w2t = [
        np.ascontiguousarray(
            w2[e].reshape(NH // 8, 8, P, D).transpose(0, 2, 1, 3)
        ).astype(_BF_NP)
        for e in range(E)
    ]
    b1r = [np.ascontiguousarray(b1[e].reshape(NH, P).T) for e in range(E)]
    b2r = [np.ascontiguousarray(b2[e].reshape(ND, P).T) for e in range(E)]

    out = np.zeros((T, D), dtype=np.float32)
    last_results = None
    for s in range(nseg):
        in_maps = []
        for e in range(E):
            sel = idx[e][s * cseg : (s + 1) * cseg]
            xg = np.zeros((cseg, D), dtype=np.float32)
            xg[: len(sel)] = xf[sel]
            # xt[p, dt, j] = xg[j, dt*P + p] — contiguous per partition
            xt = np.ascontiguousarray(
                xg.T.reshape(ND, P, cseg).transpose(1, 0, 2)
            ).astype(_BF_NP)
            in_maps.append(
                {"xt": xt, "w1t": w1t[e], "b1r": b1r[e], "w2t": w2t[e], "b2r": b2r[e]}
            )
        res = bass_utils.run_bass_kernel_spmd(
            nc,
            in_maps,
            core_ids=list(range(E)),
            trace=trace,
            trace_cores=trace_cores,
        )
        last_results = res
        for e in range(E):
            sel = idx[e][s * cseg : (s + 1) * cseg]
            if len(sel) == 0:
                continue
            yg = res.results[e]["yt"].reshape(D, cseg).T
            out[sel] = yg[: len(sel)] * coef[sel, None]

    return out.reshape(B, N, D), aux, last_results


def kernel(x, gumbel, router_w, router_b, w1, b1, w2, b2):
    out, aux, _ = _run(x, gumbel, router_w, router_b, w1, b1, w2, b2)
    return out, aux
